# revision 19
# baseline (speedup 1.0000x reference)
"""Trainium2 Bass kernel for BeliefPlausibility (Dempster-Shafer bel/pl maps).

Problem: input [4, 384, 1248, 7] fp32 (6 singleton masses + omega per pixel).
Output: tuple (bel, pl), each [4, 384, 1248, 64] fp32 where, per pixel with
masses m_0..m_5 and omega w:
    bel[q] = sum_c m_c * ((q >> c) & 1)  for q in 1..62;  bel[0]=0, bel[63]=1
    pl[q]  = bel[q] + w                  for q in 1..62;  pl[0]=0,  pl[63]=1

Strategy (pure data parallel over 8 cores, no cross-core communication):
  - Flatten pixels; each core gets 239,616 pixels as [117, 128, 112]
    (117 supertiles x 128 partitions x 16 pixels x 7 channels).
  - Per supertile: contiguous DMA in [128, 112]; PE-transpose to channels-on-
    partitions; two fp32 matmuls against a constant [112, 1024] membership
    matrix produce PSUM [128, 512] already in the per-pixel bel layout
    (8 pixel-groups x 64 output columns); ACT copies bel PSUM->SBUF; DVE
    derives pl = bel + omega with a zero-stride broadcast AP; constant
    columns 0/63 are written directly; two contiguous 512 KB DMAs per
    output store the results.
  - Walrus allows only ONE sync-wait on an fp32 (self-weight-loading)
    Matmult, so tiny "absorber" matmuls (d1/d2 into a dummy PSUM tile)
    observe the in-DMA / DVE ticks first, keeping every real Matmult at
    <=1 wait.  PSUM is read by a single engine per tensor (ACT for the
    matmul banks, DVE for the transpose bank) for the same reason.
"""

import os
import sys

import numpy as np

if "concourse" not in sys.modules:
    try:
        import concourse  # noqa: F401
    except ImportError:
        sys.path.insert(0, "/opt/trn_rl_repo")

import concourse.bacc as bacc
import concourse.bass as bass
import concourse.mybir as mybir
import concourse.tile as tile
from concourse.bass_utils import run_bass_kernel_spmd

F32 = mybir.dt.float32
ACT_COPY = mybir.ActivationFunctionType.Copy

N_CORES = 8
PX_TOTAL = 4 * 384 * 1248          # 1,916,928 pixels
PX_CORE = PX_TOTAL // N_CORES      # 239,616
PX_PART = 16                       # pixels per partition per supertile
PX_TILE = 128 * PX_PART            # 2048 pixels per supertile
N_TILES = PX_CORE // PX_TILE       # 117
N_CH = 7                           # 6 singletons + omega
N_SUB = 64                         # output positions per pixel
K_ROWS = PX_PART * N_CH            # 112 channel rows
GROUPS_PER_MM = 8                  # pixel-groups covered by one matmul
N_MM = PX_PART // GROUPS_PER_MM    # 2 matmuls per supertile


def _weight_matrix() -> np.ndarray:
    """[112, 1024]: W[7j+c, 512h+64g+q] = (q>>c)&1 for j=8h+g, q in 1..62,
    c in 0..5.  Columns (g,0) and (g,63) stay zero (written separately)."""
    w = np.zeros((K_ROWS, N_MM * 512), np.float32)
    for h in range(N_MM):
        for g in range(GROUPS_PER_MM):
            j = GROUPS_PER_MM * h + g
            col0 = 512 * h + 64 * g
            for q in range(1, 63):
                for c in range(6):
                    if (q >> c) & 1:
                        w[7 * j + c, col0 + q] = 1.0
    return w


def build_program(n_tiles: int = N_TILES, reps: int = 1) -> bass.Bass:
    # Bacc (not plain Bass): its compile() runs generate_event_semaphores,
    # which splits multi-semaphore waits into standalone event-sem
    # instructions (TRN2 allows at most one wait per instruction).
    nc = bacc.Bacc("TRN2")

    x = nc.dram_tensor("x", (n_tiles, 128, PX_PART * N_CH), F32,
                       kind="ExternalInput")
    bel = nc.dram_tensor("bel", (n_tiles, 128, PX_PART * N_SUB), F32,
                         kind="ExternalOutput")
    pl = nc.dram_tensor("pl", (n_tiles, 128, PX_PART * N_SUB), F32,
                        kind="ExternalOutput")

    w_dram = nc.inline_tensor(_weight_matrix(), name="wmat")
    id_dram = nc.inline_tensor(np.eye(128, dtype=np.float32), name="ident")

    with tile.TileContext(nc) as tc:
        with (
            tc.tile_pool(name="const", bufs=1) as cpool,
            tc.tile_pool(name="inp", bufs=4) as inpool,
            tc.tile_pool(name="tp", bufs=3) as tpool,
            tc.tile_pool(name="om", bufs=3) as ompool,
            tc.tile_pool(name="outb", bufs=3) as belpool,
            tc.tile_pool(name="outp", bufs=3) as plpool,
            tc.tile_pool(name="psT", bufs=2, space="PSUM") as psTpool,
            tc.tile_pool(name="psM", bufs=1, space="PSUM") as psMpool,
            tc.tile_pool(name="psD", bufs=1, space="PSUM") as psDpool,
        ):
            # Stage the constants through an ACT copy: matmuls reading an
            # ACT-produced tensor can merge that dep with their other ACT
            # deps into a single semaphore wait (walrus allows only one
            # sync-wait on fp32 Matmults).
            wstage = cpool.tile([K_ROWS, N_MM * 512], F32)
            nc.sync.dma_start(wstage[:], w_dram[:])
            wmat = cpool.tile([K_ROWS, N_MM * 512], F32)
            nc.scalar.copy(wmat[:], wstage[:])
            istage = cpool.tile([128, 128], F32)
            nc.sync.dma_start(istage[:], id_dram[:])
            ident = cpool.tile([128, 128], F32)
            nc.scalar.copy(ident[:], istage[:])
            dum = psDpool.tile([1, 1], F32)
            # One persistent 4-bank PSUM tensor, slices cycled manually:
            # avoids pool-release machinery so matmul slot-reuse deps stay
            # byte-range (same-engine WAW = program order, reader WAR = ACT).
            ps_all = psMpool.tile([128, 4 * 512], F32)

            for g in range(reps * n_tiles):
                t = g % n_tiles
                in_tile = inpool.tile([128, K_ROWS], F32)
                nc.sync.dma_start(in_tile[:], x[t])

                # d1: absorb the in-DMA wait on PE so the transpose
                # (an fp32 Matmult, max one sync-wait) stays at <=1 wait.
                nc.tensor.matmul(dum[:], in_tile[0:1, 0:1], in_tile[0:1, 0:1])

                ps_t = psTpool.tile([K_ROWS, 128], F32)
                nc.tensor.transpose(ps_t[:], in_tile[:], ident[:])

                # `that` is produced on ACT so the matmuls' two deps (data
                # RAW + PSUM-slot release, whose reader is also ACT) merge
                # into a single ACT semaphore wait.
                that = tpool.tile([K_ROWS, 128], F32)
                nc.scalar.copy(that[:], ps_t[:])

                # Stage the omega channels through DVE: the pl tensor_add
                # then reads only DVE- and ACT-produced operands, keeping
                # it at one semaphore wait (ISA limit per instruction).
                omg = ompool.tile([128, PX_PART], F32)
                nc.vector.tensor_copy(omg[:], in_tile[:, 6:K_ROWS:7])

                bel_t = belpool.tile([128, PX_PART * N_SUB], F32)
                pl_t = plpool.tile([128, PX_PART * N_SUB], F32)
                bel3 = bel_t[:].rearrange("p (g q) -> p g q", q=N_SUB)
                pl3 = pl_t[:].rearrange("p (g q) -> p g q", q=N_SUB)

                # constant columns: bel/pl col 63 = 1, pl col 0 = 0
                # (bel col 0 comes from the all-zero W column via the copy).
                nc.scalar.activation(bel3[:, :, 63:64], ident[:, 0:PX_PART],
                                     ACT_COPY, bias=1.0, scale=0.0)
                nc.vector.memset(pl3[:, :, 0:1], 0.0)
                nc.vector.memset(pl3[:, :, 63:64], 1.0)

                for h in range(N_MM):
                    slot = (2 * g + h) % 4
                    ps = ps_all[:, 512 * slot:512 * (slot + 1)]
                    nc.tensor.matmul(
                        ps, that[:], wmat[:, 512 * h:512 * (h + 1)])
                    ps3 = ps.rearrange("p (g q) -> p g q", q=N_SUB)
                    gsl = slice(GROUPS_PER_MM * h, GROUPS_PER_MM * (h + 1))

                    # bel columns 0..62 of each group: ACT copy PSUM->SBUF
                    nc.scalar.copy(bel3[:, gsl, 0:63], ps3[:, :, 0:63])

                    # pl columns 1..62: bel + omega (zero-stride broadcast)
                    om = omg[:, GROUPS_PER_MM * h:GROUPS_PER_MM * (h + 1)]
                    om = bass.AP(om.tensor, om.offset, om.ap + [[0, 62]])
                    nc.vector.tensor_add(pl3[:, gsl, 1:63],
                                         bel3[:, gsl, 1:63], om)

                nc.sync.dma_start(bel[t], bel_t[:])
                nc.sync.dma_start(pl[t], pl_t[:])

    nc.compile()
    return nc


_NC_CACHE: dict[int, bass.Bass] = {}


def _get_program(n_tiles: int) -> bass.Bass:
    if n_tiles not in _NC_CACHE:
        _NC_CACHE[n_tiles] = build_program(n_tiles)
    return _NC_CACHE[n_tiles]


def run_on_cores(x_flat: np.ndarray, **run_kwargs):
    """x_flat: [PX_TOTAL, 7] fp32. Returns (bel, pl) each [PX_TOTAL, 64],
    plus the raw BassKernelResults as third element."""
    nc = _get_program(N_TILES)
    in_maps = []
    for c in range(N_CORES):
        shard = np.ascontiguousarray(
            x_flat[c * PX_CORE:(c + 1) * PX_CORE]).reshape(
                N_TILES, 128, PX_PART * N_CH)
        in_maps.append({"x": shard})
    rr = run_bass_kernel_spmd(nc, in_maps, core_ids=list(range(N_CORES)),
                              **run_kwargs)
    bel = np.empty((PX_TOTAL, N_SUB), np.float32)
    pl = np.empty((PX_TOTAL, N_SUB), np.float32)
    for c, res in enumerate(rr.results):
        sl = slice(c * PX_CORE, (c + 1) * PX_CORE)
        bel[sl] = np.asarray(res["bel"]).reshape(PX_CORE, N_SUB)
        pl[sl] = np.asarray(res["pl"]).reshape(PX_CORE, N_SUB)
    return bel, pl, rr


def kernel(inputs: np.ndarray):
    inputs = np.ascontiguousarray(np.asarray(inputs, dtype=np.float32))
    b, hh, ww, ch = inputs.shape
    x_flat = inputs.reshape(-1, ch)
    bel, pl, _ = run_on_cores(x_flat)
    return (bel.reshape(b, hh, ww, N_SUB), pl.reshape(b, hh, ww, N_SUB))


# revision 23
# speedup vs baseline: 1.1085x; 1.1085x over previous
"""Trainium2 Bass kernel for BeliefPlausibility (Dempster-Shafer bel/pl maps).

Problem: input [4, 384, 1248, 7] fp32 (6 singleton masses + omega per pixel).
Output: tuple (bel, pl), each [4, 384, 1248, 64] fp32 where, per pixel with
masses m_0..m_5 and omega w:
    bel[q] = sum_c m_c * ((q >> c) & 1)  for q in 1..62;  bel[0]=0, bel[63]=1
    pl[q]  = bel[q] + w                  for q in 1..62;  pl[0]=0,  pl[63]=1

Strategy (pure data parallel over 8 cores, no cross-core communication):
  - Flatten pixels; each core gets 239,616 pixels as [117, 128, 112]
    (117 supertiles x 128 partitions x 16 pixels x 7 channels).
  - Per supertile: contiguous DMA in [128, 112]; PE-transpose to channels-on-
    partitions; two fp32 matmuls against a constant [112, 1024] membership
    matrix produce PSUM [128, 512] already in the per-pixel bel layout
    (8 pixel-groups x 64 output columns); ACT copies bel PSUM->SBUF; DVE
    derives pl = bel + omega with a zero-stride broadcast AP; constant
    columns 0/63 are written directly; two contiguous 512 KB DMAs per
    output store the results.
  - Walrus allows only ONE sync-wait on an fp32 (self-weight-loading)
    Matmult, so tiny "absorber" matmuls (d1/d2 into a dummy PSUM tile)
    observe the in-DMA / DVE ticks first, keeping every real Matmult at
    <=1 wait.  PSUM is read by a single engine per tensor (ACT for the
    matmul banks, DVE for the transpose bank) for the same reason.
"""

import os
import sys

import numpy as np

if "concourse" not in sys.modules:
    try:
        import concourse  # noqa: F401
    except ImportError:
        sys.path.insert(0, "/opt/trn_rl_repo")

import concourse.bacc as bacc
import concourse.bass as bass
import concourse.mybir as mybir
import concourse.tile as tile
from concourse.bass_utils import run_bass_kernel_spmd

F32 = mybir.dt.float32
F32R = mybir.dt.float32r
ACT_COPY = mybir.ActivationFunctionType.Copy

N_CORES = 8
PX_TOTAL = 4 * 384 * 1248          # 1,916,928 pixels
PX_CORE = PX_TOTAL // N_CORES      # 239,616
PX_PART = 16                       # pixels per partition per supertile
PX_TILE = 128 * PX_PART            # 2048 pixels per supertile
N_TILES = PX_CORE // PX_TILE       # 117
N_CH = 7                           # 6 singletons + omega
N_SUB = 64                         # output positions per pixel
K_ROWS = PX_PART * N_CH            # 112 channel rows
GROUPS_PER_MM = 8                  # pixel-groups covered by one matmul
N_MM = PX_PART // GROUPS_PER_MM    # 2 matmuls per supertile


def _weight_matrix() -> np.ndarray:
    """[112, 1024]: W[7j+c, 512h+64g+q] = (q>>c)&1 for j=8h+g, q in 1..62,
    c in 0..5.  Columns (g,0) and (g,63) stay zero (written separately)."""
    w = np.zeros((K_ROWS, N_MM * 512), np.float32)
    for h in range(N_MM):
        for g in range(GROUPS_PER_MM):
            j = GROUPS_PER_MM * h + g
            col0 = 512 * h + 64 * g
            for q in range(1, 63):
                for c in range(6):
                    if (q >> c) & 1:
                        w[7 * j + c, col0 + q] = 1.0
    return w


def build_program(n_tiles: int = N_TILES, reps: int = 1,
                  use_f32r: bool = False) -> bass.Bass:
    # Bacc (not plain Bass): its compile() runs generate_event_semaphores,
    # which splits multi-semaphore waits into standalone event-sem
    # instructions (TRN2 allows at most one wait per instruction).
    nc = bacc.Bacc("TRN2")

    x = nc.dram_tensor("x", (n_tiles, 128, PX_PART * N_CH), F32,
                       kind="ExternalInput")
    bel = nc.dram_tensor("bel", (n_tiles, 128, PX_PART * N_SUB), F32,
                         kind="ExternalOutput")
    pl = nc.dram_tensor("pl", (n_tiles, 128, PX_PART * N_SUB), F32,
                        kind="ExternalOutput")

    w_dram = nc.inline_tensor(_weight_matrix(), name="wmat")
    id_dram = nc.inline_tensor(np.eye(128, dtype=np.float32), name="ident")

    with tile.TileContext(nc) as tc:
        with (
            tc.tile_pool(name="const", bufs=1) as cpool,
            tc.tile_pool(name="inp", bufs=8) as inpool,
            tc.tile_pool(name="tp", bufs=4) as tpool,
            tc.tile_pool(name="om", bufs=4) as ompool,
            tc.tile_pool(name="outb", bufs=4) as belpool,
            tc.tile_pool(name="outp", bufs=4) as plpool,
            tc.tile_pool(name="psT", bufs=3, space="PSUM") as psTpool,
            tc.tile_pool(name="psM", bufs=1, space="PSUM") as psMpool,
            tc.tile_pool(name="psD", bufs=1, space="PSUM") as psDpool,
        ):
            # Stage the constants through an ACT copy: matmuls reading an
            # ACT-produced tensor can merge that dep with their other ACT
            # deps into a single semaphore wait (walrus allows only one
            # sync-wait on fp32 Matmults).
            wstage = cpool.tile([K_ROWS, N_MM * 512], F32)
            nc.sync.dma_start(wstage[:], w_dram[:])
            wmat = cpool.tile([K_ROWS, N_MM * 512], F32)
            nc.scalar.copy(wmat[:], wstage[:])
            istage = cpool.tile([128, 128], F32)
            nc.sync.dma_start(istage[:], id_dram[:])
            ident = cpool.tile([128, 128], F32)
            nc.scalar.copy(ident[:], istage[:])
            dum = psDpool.tile([1, 1], F32)
            # One persistent 4-bank PSUM tensor, slices cycled manually:
            # avoids pool-release machinery so matmul slot-reuse deps stay
            # byte-range (same-engine WAW = program order, reader WAR = ACT).
            ps_all = psMpool.tile([128, 4 * 512], F32)

            for g in range(reps * n_tiles):
                t = g % n_tiles
                in_tile = inpool.tile([128, K_ROWS], F32)
                nc.sync.dma_start(in_tile[:], x[t])

                # d1: absorb the in-DMA wait on PE so the transpose
                # (an fp32 Matmult, max one sync-wait) stays at <=1 wait.
                nc.tensor.matmul(dum[:], in_tile[0:1, 0:1], in_tile[0:1, 0:1])

                ps_t = psTpool.tile([K_ROWS, 128], F32)
                nc.tensor.transpose(ps_t[:], in_tile[:], ident[:])

                # `that` is produced on ACT so the matmuls' two deps (data
                # RAW + PSUM-slot release, whose reader is also ACT) merge
                # into a single ACT semaphore wait.
                that = tpool.tile([K_ROWS, 128], F32)
                nc.scalar.copy(that[:], ps_t[:])

                # Stage the omega channels through DVE: the pl tensor_add
                # then reads only DVE- and ACT-produced operands, keeping
                # it at one semaphore wait (ISA limit per instruction).
                omg = ompool.tile([128, PX_PART], F32)
                nc.vector.tensor_copy(omg[:], in_tile[:, 6:K_ROWS:7])

                bel_t = belpool.tile([128, PX_PART * N_SUB], F32)
                pl_t = plpool.tile([128, PX_PART * N_SUB], F32)
                bel3 = bel_t[:].rearrange("p (g q) -> p g q", q=N_SUB)
                pl3 = pl_t[:].rearrange("p (g q) -> p g q", q=N_SUB)

                # constant columns: bel/pl col 63 = 1, pl col 0 = 0
                # (bel col 0 comes from the all-zero W column via the copy).
                nc.scalar.activation(bel3[:, :, 63:64], ident[:, 0:PX_PART],
                                     ACT_COPY, bias=1.0, scale=0.0)
                nc.vector.memset(pl3[:, :, 0:1], 0.0)
                nc.vector.memset(pl3[:, :, 63:64], 1.0)

                for h in range(N_MM):
                    slot = (2 * g + h) % 4
                    ps = ps_all[:, 512 * slot:512 * (slot + 1)]
                    lhsT = that[:]
                    rhs = wmat[:, 512 * h:512 * (h + 1)]
                    if use_f32r:
                        lhsT = lhsT.bitcast(F32R)
                        rhs = rhs.bitcast(F32R)
                    nc.tensor.matmul(ps, lhsT, rhs)
                    ps3 = ps.rearrange("p (g q) -> p g q", q=N_SUB)
                    gsl = slice(GROUPS_PER_MM * h, GROUPS_PER_MM * (h + 1))

                    # bel columns 0..62 of each group: ACT copy PSUM->SBUF
                    nc.scalar.copy(bel3[:, gsl, 0:63], ps3[:, :, 0:63])

                    # pl columns 1..62: bel + omega (zero-stride broadcast)
                    om = omg[:, GROUPS_PER_MM * h:GROUPS_PER_MM * (h + 1)]
                    om = bass.AP(om.tensor, om.offset, om.ap + [[0, 62]])
                    nc.vector.tensor_add(pl3[:, gsl, 1:63],
                                         bel3[:, gsl, 1:63], om)

                nc.sync.dma_start(bel[t], bel_t[:])
                nc.sync.dma_start(pl[t], pl_t[:])

    nc.compile()
    return nc


_NC_CACHE: dict[int, bass.Bass] = {}


def _get_program(n_tiles: int) -> bass.Bass:
    if n_tiles not in _NC_CACHE:
        _NC_CACHE[n_tiles] = build_program(n_tiles)
    return _NC_CACHE[n_tiles]


def run_on_cores(x_flat: np.ndarray, **run_kwargs):
    """x_flat: [PX_TOTAL, 7] fp32. Returns (bel, pl) each [PX_TOTAL, 64],
    plus the raw BassKernelResults as third element."""
    nc = _get_program(N_TILES)
    in_maps = []
    for c in range(N_CORES):
        shard = np.ascontiguousarray(
            x_flat[c * PX_CORE:(c + 1) * PX_CORE]).reshape(
                N_TILES, 128, PX_PART * N_CH)
        in_maps.append({"x": shard})
    rr = run_bass_kernel_spmd(nc, in_maps, core_ids=list(range(N_CORES)),
                              **run_kwargs)
    bel = np.empty((PX_TOTAL, N_SUB), np.float32)
    pl = np.empty((PX_TOTAL, N_SUB), np.float32)
    for c, res in enumerate(rr.results):
        sl = slice(c * PX_CORE, (c + 1) * PX_CORE)
        bel[sl] = np.asarray(res["bel"]).reshape(PX_CORE, N_SUB)
        pl[sl] = np.asarray(res["pl"]).reshape(PX_CORE, N_SUB)
    return bel, pl, rr


def kernel(inputs: np.ndarray):
    inputs = np.ascontiguousarray(np.asarray(inputs, dtype=np.float32))
    b, hh, ww, ch = inputs.shape
    x_flat = inputs.reshape(-1, ch)
    bel, pl, _ = run_on_cores(x_flat)
    return (bel.reshape(b, hh, ww, N_SUB), pl.reshape(b, hh, ww, N_SUB))


# revision 26
# speedup vs baseline: 1.6647x; 1.5017x over previous
"""Trainium2 Bass kernel for BeliefPlausibility (Dempster-Shafer bel/pl maps).

Problem: input [4, 384, 1248, 7] fp32 (6 singleton masses + omega per pixel).
Output: tuple (bel, pl), each [4, 384, 1248, 64] fp32 where, per pixel with
masses m_0..m_5 and omega w:
    bel[q] = sum_c m_c * ((q >> c) & 1)  for q in 1..62;  bel[0]=0, bel[63]=1
    pl[q]  = bel[q] + w                  for q in 1..62;  pl[0]=0,  pl[63]=1

Strategy (pure data parallel over 8 cores, no cross-core communication):
  - Flatten pixels; each core gets 239,616 pixels as [117, 128, 112]
    (117 supertiles x 128 partitions x 16 pixels x 7 channels).
  - Per supertile: contiguous DMA in [128, 112]; PE-transpose to channels-on-
    partitions; two fp32 matmuls against a constant [112, 1024] membership
    matrix produce PSUM [128, 512] already in the per-pixel bel layout
    (8 pixel-groups x 64 output columns); ACT copies bel PSUM->SBUF; DVE
    derives pl = bel + omega with a zero-stride broadcast AP; constant
    columns 0/63 are written directly; two contiguous 512 KB DMAs per
    output store the results.
  - Walrus allows only ONE sync-wait on an fp32 (self-weight-loading)
    Matmult, so tiny "absorber" matmuls (d1/d2 into a dummy PSUM tile)
    observe the in-DMA / DVE ticks first, keeping every real Matmult at
    <=1 wait.  PSUM is read by a single engine per tensor (ACT for the
    matmul banks, DVE for the transpose bank) for the same reason.
"""

import os
import sys

import numpy as np

if "concourse" not in sys.modules:
    try:
        import concourse  # noqa: F401
    except ImportError:
        sys.path.insert(0, "/opt/trn_rl_repo")

import concourse.bacc as bacc
import concourse.bass as bass
import concourse.mybir as mybir
import concourse.tile as tile
from concourse.bass_utils import run_bass_kernel_spmd

F32 = mybir.dt.float32
F32R = mybir.dt.float32r
ACT_COPY = mybir.ActivationFunctionType.Copy

N_CORES = 8
PX_TOTAL = 4 * 384 * 1248          # 1,916,928 pixels
PX_CORE = PX_TOTAL // N_CORES      # 239,616
PX_PART = 16                       # pixels per partition per supertile
PX_TILE = 128 * PX_PART            # 2048 pixels per supertile
N_TILES = PX_CORE // PX_TILE       # 117
N_CH = 7                           # 6 singletons + omega
N_SUB = 64                         # output positions per pixel
K_ROWS = PX_PART * N_CH            # 112 channel rows
GROUPS_PER_MM = 8                  # pixel-groups covered by one matmul
N_MM = PX_PART // GROUPS_PER_MM    # 2 matmuls per supertile


def _weight_matrix() -> np.ndarray:
    """[112, 1024]: W[7j+c, 512h+64g+q] = (q>>c)&1 for j=8h+g, q in 1..62,
    c in 0..5.  Columns (g,0) and (g,63) stay zero (written separately)."""
    w = np.zeros((K_ROWS, N_MM * 512), np.float32)
    for h in range(N_MM):
        for g in range(GROUPS_PER_MM):
            j = GROUPS_PER_MM * h + g
            col0 = 512 * h + 64 * g
            for q in range(1, 63):
                for c in range(6):
                    if (q >> c) & 1:
                        w[7 * j + c, col0 + q] = 1.0
    return w


def build_program(n_tiles: int = N_TILES, reps: int = 1,
                  use_f32r: bool = False) -> bass.Bass:
    # Bacc (not plain Bass): its compile() runs generate_event_semaphores,
    # which splits multi-semaphore waits into standalone event-sem
    # instructions (TRN2 allows at most one wait per instruction).
    nc = bacc.Bacc("TRN2")

    x = nc.dram_tensor("x", (n_tiles, 128, PX_PART * N_CH), F32,
                       kind="ExternalInput")
    bel = nc.dram_tensor("bel", (n_tiles, 128, PX_PART * N_SUB), F32,
                         kind="ExternalOutput")
    pl = nc.dram_tensor("pl", (n_tiles, 128, PX_PART * N_SUB), F32,
                        kind="ExternalOutput")

    w_dram = nc.inline_tensor(_weight_matrix(), name="wmat")
    id_dram = nc.inline_tensor(np.eye(128, dtype=np.float32), name="ident")

    with tile.TileContext(nc) as tc:
        with (
            tc.tile_pool(name="const", bufs=1) as cpool,
            tc.tile_pool(name="inp", bufs=8) as inpool,
            tc.tile_pool(name="tp", bufs=4) as tpool,
            tc.tile_pool(name="om", bufs=4) as ompool,
            tc.tile_pool(name="outb", bufs=4) as belpool,
            tc.tile_pool(name="outp", bufs=4) as plpool,
            tc.tile_pool(name="psT", bufs=3, space="PSUM") as psTpool,
            tc.tile_pool(name="psM", bufs=1, space="PSUM") as psMpool,
            tc.tile_pool(name="psD", bufs=1, space="PSUM") as psDpool,
        ):
            # Stage the constants through an ACT copy: matmuls reading an
            # ACT-produced tensor can merge that dep with their other ACT
            # deps into a single semaphore wait (walrus allows only one
            # sync-wait on fp32 Matmults).
            mm_dt = F32R if use_f32r else F32
            wstage = cpool.tile([K_ROWS, N_MM * 512], F32)
            nc.sync.dma_start(wstage[:], w_dram[:])
            wmat = cpool.tile([K_ROWS, N_MM * 512], mm_dt)
            nc.scalar.copy(wmat[:], wstage[:])
            istage = cpool.tile([128, 128], F32)
            nc.sync.dma_start(istage[:], id_dram[:])
            ident = cpool.tile([128, 128], F32)
            nc.scalar.copy(ident[:], istage[:])
            dum = psDpool.tile([1, 1], F32)
            # One persistent 4-bank PSUM tensor, slices cycled manually:
            # avoids pool-release machinery so matmul slot-reuse deps stay
            # byte-range (same-engine WAW = program order, reader WAR = ACT).
            ps_all = psMpool.tile([128, 4 * 512], F32)

            for g in range(reps * n_tiles):
                t = g % n_tiles
                in_tile = inpool.tile([128, K_ROWS], F32)
                nc.sync.dma_start(in_tile[:], x[t])

                # d1: absorb the in-DMA wait on PE so the transpose
                # (an fp32 Matmult, max one sync-wait) stays at <=1 wait.
                nc.tensor.matmul(dum[:], in_tile[0:1, 0:1], in_tile[0:1, 0:1])

                ps_t = psTpool.tile([K_ROWS, 128], F32)
                nc.tensor.transpose(ps_t[:], in_tile[:], ident[:])

                # `that` is produced on ACT so the matmuls' two deps (data
                # RAW + PSUM-slot release, whose reader is also ACT) merge
                # into a single ACT semaphore wait.
                that = tpool.tile([K_ROWS, 128], mm_dt)
                nc.scalar.copy(that[:], ps_t[:])

                # Stage the omega channels through DVE: the pl tensor_add
                # then reads only DVE- and ACT-produced operands, keeping
                # it at one semaphore wait (ISA limit per instruction).
                omg = ompool.tile([128, PX_PART], F32)
                nc.vector.tensor_copy(omg[:], in_tile[:, 6:K_ROWS:7])

                bel_t = belpool.tile([128, PX_PART * N_SUB], F32)
                pl_t = plpool.tile([128, PX_PART * N_SUB], F32)
                bel3 = bel_t[:].rearrange("p (g q) -> p g q", q=N_SUB)
                pl3 = pl_t[:].rearrange("p (g q) -> p g q", q=N_SUB)

                # constant columns: bel/pl col 63 = 1, pl col 0 = 0
                # (bel col 0 comes from the all-zero W column via the copy).
                nc.scalar.activation(bel3[:, :, 63:64], ident[:, 0:PX_PART],
                                     ACT_COPY, bias=1.0, scale=0.0)
                nc.vector.memset(pl3[:, :, 0:1], 0.0)
                nc.vector.memset(pl3[:, :, 63:64], 1.0)

                for h in range(N_MM):
                    slot = (2 * g + h) % 4
                    ps = ps_all[:, 512 * slot:512 * (slot + 1)]
                    nc.tensor.matmul(ps, that[:],
                                     wmat[:, 512 * h:512 * (h + 1)])
                    ps3 = ps.rearrange("p (g q) -> p g q", q=N_SUB)
                    gsl = slice(GROUPS_PER_MM * h, GROUPS_PER_MM * (h + 1))

                    # bel columns 0..62 of each group: ACT copy PSUM->SBUF
                    nc.scalar.copy(bel3[:, gsl, 0:63], ps3[:, :, 0:63])

                    # pl columns 1..62: bel + omega (zero-stride broadcast)
                    om = omg[:, GROUPS_PER_MM * h:GROUPS_PER_MM * (h + 1)]
                    om = bass.AP(om.tensor, om.offset, om.ap + [[0, 62]])
                    nc.vector.tensor_add(pl3[:, gsl, 1:63],
                                         bel3[:, gsl, 1:63], om)

                nc.sync.dma_start(bel[t], bel_t[:])
                nc.sync.dma_start(pl[t], pl_t[:])

    nc.compile()
    return nc


_NC_CACHE: dict[int, bass.Bass] = {}


def _get_program(n_tiles: int) -> bass.Bass:
    if n_tiles not in _NC_CACHE:
        _NC_CACHE[n_tiles] = build_program(n_tiles)
    return _NC_CACHE[n_tiles]


def run_on_cores(x_flat: np.ndarray, **run_kwargs):
    """x_flat: [PX_TOTAL, 7] fp32. Returns (bel, pl) each [PX_TOTAL, 64],
    plus the raw BassKernelResults as third element."""
    nc = _get_program(N_TILES)
    in_maps = []
    for c in range(N_CORES):
        shard = np.ascontiguousarray(
            x_flat[c * PX_CORE:(c + 1) * PX_CORE]).reshape(
                N_TILES, 128, PX_PART * N_CH)
        in_maps.append({"x": shard})
    rr = run_bass_kernel_spmd(nc, in_maps, core_ids=list(range(N_CORES)),
                              **run_kwargs)
    bel = np.empty((PX_TOTAL, N_SUB), np.float32)
    pl = np.empty((PX_TOTAL, N_SUB), np.float32)
    for c, res in enumerate(rr.results):
        sl = slice(c * PX_CORE, (c + 1) * PX_CORE)
        bel[sl] = np.asarray(res["bel"]).reshape(PX_CORE, N_SUB)
        pl[sl] = np.asarray(res["pl"]).reshape(PX_CORE, N_SUB)
    return bel, pl, rr


def kernel(inputs: np.ndarray):
    inputs = np.ascontiguousarray(np.asarray(inputs, dtype=np.float32))
    b, hh, ww, ch = inputs.shape
    x_flat = inputs.reshape(-1, ch)
    bel, pl, _ = run_on_cores(x_flat)
    return (bel.reshape(b, hh, ww, N_SUB), pl.reshape(b, hh, ww, N_SUB))
